# revision 3
# baseline (speedup 1.0000x reference)
"""Trainium2 Bass kernel for nn_Block_9345848836513.

Pipeline (per (batch, half-of-channels) core; 8 cores total):
  1. transposed channel mix: y^T produced directly in time-on-partitions
     layout via PE matmuls with the x-chunk as the stationary operand
     (lhsT = x[c, 128 t's], rhs = mixer columns) -> BigX[p, j*16+dd] =
     y[dd, j*128+p]; no HBM roundtrip, no separate transpose pass
  2. forward rfft of 512-sample frames (hop 256) as a dense real [512,512]
     matmul in float32r (full-rate PE); the 512 real DOFs are
     Re[0..256] ++ Im[1..255]; the 50%-overlap frames are read with
     strided column APs straight out of BigX (no data duplication)
  3. per-frame recurrence out_f = (spec_f + out_{f-1}) * transfer via
     tensor_tensor_scan (op0=add, op1=mult) along the frame (free) axis
  4. inverse rfft with the Hann window folded into the matrix; the
     overlap-add is folded into PSUM accumulation (second group of
     matmuls writes with a one-column shift); tanh straight from PSUM
Host only does slicing/layout reshapes; all FLOPs run on device.
"""

import numpy as np

import concourse.bass as bass
import concourse.mybir as mybir
import concourse.tile as tile
from concourse import bacc
from concourse.bass_utils import run_bass_kernel_spmd

WINDOW = 512
HOP = 256
NCOEF = 257
NDOF = 512
B, C, T = 4, 32, 131072
F = T // HOP          # 512 frames
CPC = 16              # channels per core
NCORES = 8
JCOLS = T // 128      # 1024 output columns per channel
BXCOLS = (JCOLS + 2) * CPC  # 16416: two zero-pad j columns for the frame tail
FP32 = mybir.dt.float32
FP32R = mybir.dt.float32r


def _build_dft_matrices():
    w = np.arange(WINDOW)
    k = np.arange(NCOEF)
    ang = 2.0 * np.pi * np.outer(w, k) / WINDOW
    cos, sin = np.cos(ang), np.sin(ang)
    fmat = np.zeros((WINDOW, NDOF), np.float64)
    fmat[:, :NCOEF] = cos
    fmat[:, NCOEF:] = -sin[:, 1:256]
    hann = 0.5 - 0.5 * np.cos(2.0 * np.pi * w / WINDOW)
    g = np.zeros((NDOF, WINDOW), np.float64)
    g[0, :] = 1.0
    g[256, :] = cos[:, 256]
    for kk in range(1, 256):
        g[kk, :] = 2.0 * cos[:, kk]
        g[256 + kk, :] = -2.0 * sin[:, kk]
    g *= hann[None, :] / WINDOW
    # radix-2 dof permutation: [Re even k] [Im even k] [Re odd k] [Im odd k].
    # fmat rows 256..512 at even/odd k equal +/- rows 0..256, so the whole
    # forward DFT folds to e/o = fr[p] +/- fr[p+256] and fmat[0:256, perm].
    perm = ([2 * kk for kk in range(129)]
            + [256 + 2 * kk for kk in range(1, 128)]
            + [2 * kk + 1 for kk in range(128)]
            + [256 + 2 * kk + 1 for kk in range(128)])
    perm = np.asarray(perm)
    f2 = fmat[0:256, :][:, perm]                     # [256, 512]
    g2 = g[perm, :]                                  # [512, 512]
    # [p, chunk_row, chunk_col, j] layouts for [128,128] lhsT blocks
    f_l = f2.reshape(2, 128, 4, 128).transpose(1, 0, 2, 3)
    g_l = g2.reshape(4, 128, 4, 128).transpose(1, 0, 2, 3)
    return (np.ascontiguousarray(f_l, np.float32),
            np.ascontiguousarray(g_l, np.float32), perm)


def _build_program(detect_races=True, reps=1):
    nc = bacc.Bacc("TRN2", target_bir_lowering=False, num_devices=NCORES,
                   detect_race_conditions=detect_races)
    # xq[32*q + c, tau] = x[c, q*(T//4) + tau]  (host pre-permuted)
    xq = nc.dram_tensor("xq", [128, T // 4], FP32, kind="ExternalInput")
    mixrep = nc.dram_tensor("mixrep", [128, CPC], FP32, kind="ExternalInput")
    fmat = nc.dram_tensor("fmat", [128, 2, 4, 128], FP32, kind="ExternalInput")
    gmat = nc.dram_tensor("gmat", [128, 4, 4, 128], FP32, kind="ExternalInput")
    trt = nc.dram_tensor("trt", [128, CPC * 4], FP32, kind="ExternalInput")
    gainv = nc.dram_tensor("gainv", [1, CPC], FP32, kind="ExternalInput")
    out_d = nc.dram_tensor("out", [CPC, 128, JCOLS], FP32, kind="ExternalOutput")

    ADD, MUL = mybir.AluOpType.add, mybir.AluOpType.mult

    with tile.TileContext(nc) as tc:
        with tc.tile_pool(name="singles", bufs=1) as singles:
            fsb = singles.tile([128, 2, 4, 128], FP32R)
            gsb = singles.tile([128, 4, 4, 128], FP32R)
            mix_sb = singles.tile([128, CPC], FP32R)
            trsb = singles.tile([128, CPC * 4], FP32)
            gain_sb = singles.tile([128, CPC], FP32)
            bigx = singles.tile([128, BXCOLS], FP32R)
            u_all = [singles.tile([128, 513], FP32R, name=f"uall{i}")
                     for i in range(8)]
            for ut in u_all:
                nc.vector.memset(ut[:, 0:1].bitcast(FP32), 0.0)
            # params ride the SWDGE queue so the first phase-A x-load
            # (HWDGE) isn't queued behind ~1.5MB of parameter DMAs
            nc.sync.dma_start(out=mix_sb[:], in_=mixrep[:].bitcast(FP32R))
            nc.gpsimd.dma_start(out=fsb[:], in_=fmat[:].bitcast(FP32R))
            nc.gpsimd.dma_start(out=gsb[:], in_=gmat[:].bitcast(FP32R))
            nc.gpsimd.dma_start(out=trsb[:], in_=trt[:])
            nc.gpsimd.dma_start(out=gain_sb[:], in_=gainv[:].to_broadcast((128, CPC)))
            nc.vector.memset(bigx[:, JCOLS * CPC:BXCOLS].bitcast(FP32), 0.0)

            for _rep in range(reps):
                # ---- Phase A: transposed mix into BigX ----
                # xt tile i holds tau in [i*4096, (i+1)*4096) for all 4
                # quarters; matmul j covers t = j*128..j*128+128 with
                # lhsT = x[c, t-range] (stationary), rhs = mixer cols.
                with (
                    tc.tile_pool(name="xa", bufs=3) as xa,
                    tc.tile_pool(name="pmix", bufs=2, space="PSUM") as pmix,
                ):
                    for i in range(8):
                        xt = xa.tile([128, 4096], FP32R)
                        nc.sync.dma_start(
                            out=xt[:],
                            in_=xq[:, 4096 * i:4096 * (i + 1)].bitcast(FP32R),
                        )
                        pss = [pmix.tile([128, 512], FP32, name=f"psm{q}", tag=f"ps{q}")
                               for q in range(4)]
                        for jj in range(32):
                            for q in range(4):
                                nc.tensor.matmul(
                                    pss[q][:, 16 * jj:16 * (jj + 1)],
                                    lhsT=xt[32 * q:32 * (q + 1),
                                            128 * jj:128 * (jj + 1)],
                                    rhs=mix_sb[32 * q:32 * (q + 1), :],
                                    tile_position=(32 * q, 0),
                                )
                        for q in range(4):
                            blk = 8 * q + i
                            nc.scalar.copy(
                                bigx[:, 512 * blk:512 * (blk + 1)], pss[q][:])

                # ---- Phase B: DFT -> scan -> inverse+OLA -> tanh ----
                # bxv[p, parity, dd, f] = BigX col j = 2f+parity, channel dd
                bxv = bigx[:].rearrange("p (f two dd) -> p two dd f",
                                        two=2, dd=CPC)
                with (
                    tc.tile_pool(name="eo", bufs=8) as eop,
                    tc.tile_pool(name="rp", bufs=3) as rp,
                    tc.tile_pool(name="psp", bufs=4, space="PSUM") as psp,
                    tc.tile_pool(name="pwp", bufs=4, space="PSUM") as pwp,
                ):
                    SUB = mybir.AluOpType.subtract
                    for d in range(CPC):
                        # radix-2 fold: e/o[p + 128h] = fr[p+128h] +/- fr[p+128h+256]
                        # = BigX col (2f+h) +/- col (2f+h+2), per w-half h
                        eo = []
                        for isodd in range(2):
                            for h in range(2):
                                t_ = eop.tile([128, 512], FP32R, name=f"eot{isodd}{h}", tag=f"eo{isodd}{h}")
                                nc.vector.tensor_tensor(
                                    t_[:],
                                    bxv[:, h, d, 0:512].bitcast(FP32),
                                    bxv[:, h, d, 1:513].bitcast(FP32),
                                    op=SUB if isodd else mybir.AluOpType.add,
                                )
                                eo.append(t_)
                        us = []
                        for m in range(4):
                            ps = psp.tile([128, 512], FP32)
                            for h in range(2):
                                nc.tensor.matmul(
                                    ps[:],
                                    lhsT=fsb[:, h, m, :],
                                    rhs=eo[2 * (m // 2) + h][:],
                                    start=(h == 0),
                                    stop=(h == 1),
                                )
                            # col 0 zero pad keeps the shifted (f-1) OLA read
                            # 512 wide with a well-formed psum group
                            u = u_all[(d * 4 + m) % 8]
                            idx = d * 4 + m
                            nc.vector.tensor_tensor_scan(
                                u[:, 1:513], ps[:],
                                trsb[:, idx:idx + 1].broadcast_to((128, 512)),
                                0.0, op0=ADD, op1=MUL,
                            )
                            us.append(u)
                        # inverse DFT with overlap-add folded into PSUM:
                        # out col j=2f+s gets W_s[:,f] + W_{s+2}[:,f-1]
                        res = rp.tile([128, JCOLS], FP32)
                        ov = res[:].rearrange("p (f two) -> p two f", two=2)
                        for s01 in range(2):
                            pout = pwp.tile([128, 512], FP32)
                            for k in range(4):
                                nc.tensor.matmul(
                                    pout[:],
                                    lhsT=gsb[:, k, s01, :],
                                    rhs=us[k][:, 1:513],
                                    start=(k == 0),
                                    stop=False,
                                )
                            for k in range(4):
                                nc.tensor.matmul(
                                    pout[:],
                                    lhsT=gsb[:, k, s01 + 2, :],
                                    rhs=us[k][:, 0:512],
                                    start=False,
                                    stop=(k == 3),
                                )
                            nc.scalar.activation(
                                ov[:, s01, :], pout[:],
                                mybir.ActivationFunctionType.Tanh,
                                scale=gain_sb[:, d:d + 1],
                            )
                        nc.sync.dma_start(out=out_d[d], in_=res[:])
    nc.compile()
    return nc


_PROGRAM_CACHE = {}


def build_in_maps(x, transfer, mixer_matrix, gain):
    f_l, g_l, perm = _build_dft_matrices()

    # transfer per dof (re/im parts share the same real coefficient),
    # permuted into the radix-2 dof order
    tr_dof = np.empty((C, NDOF), np.float32)
    tr_dof[:, :NCOEF] = transfer
    tr_dof[:, NCOEF:] = transfer[:, 1:256]
    tr_dof = np.ascontiguousarray(tr_dof[:, perm])

    in_maps = []
    for core in range(NCORES):
        b, h = core // 2, core % 2
        d0 = h * CPC
        mixcols = mixer_matrix[:, d0:d0 + CPC]               # [32, 16]
        trd = tr_dof[d0:d0 + CPC]                            # [16, 512]
        trt = np.ascontiguousarray(
            trd.reshape(CPC, 4, 128).transpose(2, 0, 1).reshape(128, CPC * 4))
        xqv = np.ascontiguousarray(
            x[b].reshape(C, 4, T // 4).transpose(1, 0, 2).reshape(128, T // 4))
        in_maps.append({
            "xq": xqv,
            "mixrep": np.ascontiguousarray(np.tile(mixcols, (4, 1))),
            "fmat": f_l,
            "gmat": g_l,
            "trt": trt,
            "gainv": np.ascontiguousarray(gain[d0:d0 + CPC].reshape(1, CPC)),
        })
    return in_maps


def kernel(x, transfer, mixer_matrix, gain, **run_kwargs):
    x = np.ascontiguousarray(x, np.float32)
    transfer = np.asarray(transfer, np.float32)
    mixer_matrix = np.asarray(mixer_matrix, np.float32)
    gain = np.asarray(gain, np.float32)

    in_maps = build_in_maps(x, transfer, mixer_matrix, gain)

    if "nc" not in _PROGRAM_CACHE:
        _PROGRAM_CACHE["nc"] = _build_program()
    nc = _PROGRAM_CACHE["nc"]

    res = run_bass_kernel_spmd(nc, in_maps, list(range(NCORES)), **run_kwargs)

    out = np.empty((B, C, T), np.float32)
    for core in range(NCORES):
        b, h = core // 2, core % 2
        o = res.results[core]["out"]                    # [16, 128, 1024]
        out[b, h * CPC:(h + 1) * CPC] = o.transpose(0, 2, 1).reshape(CPC, T)
    kernel.last_results = res
    return out



# revision 6
# speedup vs baseline: 1.1064x; 1.1064x over previous
"""Trainium2 Bass kernel for nn_Block_9345848836513.

Per-core pipeline (8 cores = 4 batches x 2 channel-halves, 16 ch each):
  1. channel mix in fp16 on PE: lhsT = x-chunk [128 rows = 4 consecutive
     128-tau windows x 32 ch, 128 taus], rhs = block-diag 4x mixer
     [128, 64] -> psum [tau, (window, ch)]; 256 matmuls, full 128-row
     contraction, one psum tag.  x arrives from HBM as fp16 (halves the
     phase-A DMA, its critical path).  Copies scatter psum into BigX
     laid out [p, ch, j-parity, frame] (fp16) so later folds read
     contiguous rows.
  2. forward rfft of 512-sample frames (hop 256) via radix-4-folded real
     DFT: DVE folds produce C1/C2 (even-bin sources, 128 long) and
     B0/B1 (odd-bin halves); 6 fp16 matmuls per channel give all 512
     real DOFs (vs 16 for a dense [512,512]).
  3. per-frame recurrence out_i = (spec_i + out_{i-1}) * transfer via
     tensor_tensor_scan along the frame axis (fp32 state, fp16 out)
  4. inverse rfft with Hann folded into the fp16 matrix; overlap-add
     folded into PSUM accumulation (second matmul group reads with a
     one-column shift); tanh straight from PSUM.
BigX is double-buffered across reps so rep k+1's DMA/mix overlaps rep
k's transform phase; psum budget 2 (mix) + 3 (fwd) + 3 (inv) = 8 banks.
Output DMAs ride the Pool SWDGE queue, x loads the SP HWDGE queue.
"""

import numpy as np

import concourse.bass as bass
import concourse.mybir as mybir
import concourse.tile as tile
from concourse import bacc
from concourse.bass_utils import run_bass_kernel_spmd

WINDOW = 512
HOP = 256
NCOEF = 257
NDOF = 512
B, C, T = 4, 32, 131072
F = T // HOP          # 512 frames
CPC = 16              # channels per core
NCORES = 8
JCOLS = T // 128      # 1024 output columns per channel
FPAD = F + 1          # 513 frame slots per (ch, parity); last is zero pad
FP32 = mybir.dt.float32
FP32R = mybir.dt.float32r
FP16 = mybir.dt.float16
U16 = mybir.dt.uint16


def _build_dft_matrices():
    w = np.arange(WINDOW)
    k = np.arange(NCOEF)
    ang = 2.0 * np.pi * np.outer(w, k) / WINDOW
    cos, sin = np.cos(ang), np.sin(ang)
    fmat = np.zeros((WINDOW, NDOF), np.float64)
    fmat[:, :NCOEF] = cos
    fmat[:, NCOEF:] = -sin[:, 1:256]
    hann = 0.5 - 0.5 * np.cos(2.0 * np.pi * w / WINDOW)
    g = np.zeros((NDOF, WINDOW), np.float64)
    g[0, :] = 1.0
    g[256, :] = cos[:, 256]
    for kk in range(1, 256):
        g[kk, :] = 2.0 * cos[:, kk]
        g[256 + kk, :] = -2.0 * sin[:, kk]
    g *= hann[None, :] / WINDOW

    # dof indexing in the plain layout: Re k -> k (0..256), Im k -> 256+k
    def dofs_re(ks):
        return list(ks)

    def dofs_im(ks):
        return [256 + kk for kk in ks if 1 <= kk <= 255]

    # 4 chunks of 128 dofs: bins k=0 mod 4 / 2 mod 4 / 1 mod 4 / 3 mod 4.
    # Sources after the DVE folds (x split in 128-quarters x0..x3 of the
    # 512 window): C1 = x0+x1+x2+x3, C2 = x0-x1+x2-x3 (via e-halves),
    # B0 = x0-x2, B1 = x1-x3.  For k=0 mod 4: X[k] = sum_n C1[n] W^nk;
    # k=2 mod 4: C2; odd k: X[k] = sum_n B0[n] W^nk + B1[n] W^(n+128)k.
    # All coefficients are rows of the plain fmat.
    chunk_k = [
        list(range(0, 257, 4)),
        list(range(2, 256, 4)),
        list(range(1, 256, 4)),
        list(range(3, 256, 4)),
    ]
    newperm = []
    for ks in chunk_k:
        newperm += dofs_re(ks) + dofs_im(ks)
    newperm = np.asarray(newperm)
    assert newperm.size == NDOF and np.unique(newperm).size == NDOF

    p0, p1, p2, p3 = (newperm[128 * i:128 * (i + 1)] for i in range(4))
    # 6 lhsT blocks: (C1->chunk0, C2->chunk1, B0->c2, B1->c2, B0->c3, B1->c3)
    f_blocks = np.stack([
        fmat[0:128, p0],
        fmat[0:128, p1],
        fmat[0:128, p2],
        fmat[128:256, p2],
        fmat[0:128, p3],
        fmat[128:256, p3],
    ], axis=1)                                        # [128, 6, 128]
    g2 = g[newperm, :]                                # [512, 512]
    g_l = g2.reshape(4, 128, 4, 128).transpose(1, 0, 2, 3)
    return (np.ascontiguousarray(f_blocks, np.float16),
            np.ascontiguousarray(g_l, np.float16), newperm)


def _build_program(detect_races=True, reps=1):
    nc = bacc.Bacc("TRN2", target_bir_lowering=False, num_devices=NCORES,
                   detect_race_conditions=detect_races)
    # xq[32*b + c, G*128 + p] = x[c, G*512 + b*128 + p]  (fp16, host layout)
    xq = nc.dram_tensor("xq", [128, T // 4], FP16, kind="ExternalInput")
    # mix4[32*b + c, 16*b + d] = mixer[c, d0+d]; zero elsewhere (fp16)
    mix4 = nc.dram_tensor("mix4", [128, 4 * CPC], FP16, kind="ExternalInput")
    fmat = nc.dram_tensor("fmat", [128, 6, 128], FP16, kind="ExternalInput")
    gmat = nc.dram_tensor("gmat", [128, 4, 4, 128], FP16, kind="ExternalInput")
    trt = nc.dram_tensor("trt", [128, CPC * 4], FP32, kind="ExternalInput")
    gainv = nc.dram_tensor("gainv", [1, CPC], FP32, kind="ExternalInput")
    out_d = nc.dram_tensor("out", [CPC, 128, JCOLS], FP32, kind="ExternalOutput")

    ADD, MUL = mybir.AluOpType.add, mybir.AluOpType.mult
    SUB = mybir.AluOpType.subtract

    with tile.TileContext(nc) as tc:
        with (
            tc.tile_pool(name="singles", bufs=1) as singles,
            tc.tile_pool(name="xa", bufs=3) as xa,
            tc.tile_pool(name="pmix", bufs=2, space="PSUM") as pmix,
            tc.tile_pool(name="eo", bufs=3) as eop,
            tc.tile_pool(name="rp", bufs=3) as rp,
            tc.tile_pool(name="psp", bufs=3, space="PSUM") as psp,
            tc.tile_pool(name="pwp", bufs=3, space="PSUM") as pwp,
        ):
            fsb = singles.tile([128, 6, 128], FP16)
            gsb = singles.tile([128, 4, 4, 128], FP16)
            mix_sb = singles.tile([128, 4 * CPC], FP16)
            trsb = singles.tile([128, CPC * 4], FP32)
            gain_sb = singles.tile([128, CPC], FP32)
            # bigx[p, d, h, f] = y[d, t = 256f + 128h + p]  (j = 2f+h)
            bigxs = [singles.tile([128, CPC, 2, FPAD], FP16, name=f"bigx{r}")
                     for r in range(2)]
            u_all = [singles.tile([128, 513], FP16, name=f"uall{i}")
                     for i in range(8)]
            for ut in u_all:
                nc.vector.memset(ut[:, 0:1].bitcast(U16), 0)
            # params ride the SWDGE queue so the first phase-A x-load
            # (HWDGE) isn't queued behind the parameter DMAs
            nc.sync.dma_start(out=mix_sb[:], in_=mix4[:])
            nc.gpsimd.dma_start(out=fsb[:], in_=fmat[:])
            nc.gpsimd.dma_start(out=gsb[:], in_=gmat[:])
            nc.gpsimd.dma_start(out=trsb[:], in_=trt[:])
            nc.gpsimd.dma_start(out=gain_sb[:], in_=gainv[:].to_broadcast((128, CPC)))
            for bx in bigxs:
                nc.vector.memset(bx[:, :, :, F:FPAD].bitcast(U16), 0)

            for _rep in range(reps):
                bigx = bigxs[_rep % 2]
                # ---- Phase A: transposed mix into BigX (fp16) ----
                # xt tile i holds groups G in [32i, 32(i+1)); matmul for
                # group G: lhsT = xt[:, local window] -> psum cols
                # [64*G' + 16*b + d] with j = 4G + b, h = b%2,
                # f = 2*(8s + G') + b//2  (s = psum slab index 4i+g).
                for i in range(8):
                    xt = xa.tile([128, 4096], FP16, tag="xt")
                    nc.sync.dma_start(
                        out=xt[:], in_=xq[:, 4096 * i:4096 * (i + 1)])
                    for g in range(4):
                        ps = pmix.tile([128, 512], FP32, tag="pmix")
                        for gp in range(8):
                            nc.tensor.matmul(
                                ps[:, 64 * gp:64 * (gp + 1)],
                                lhsT=xt[:, 512 * g + 128 * gp:
                                        512 * g + 128 * (gp + 1)],
                                rhs=mix_sb[:],
                            )
                        # psum [p, (G' b d)] -> bigx[p, d, h, f0 + 2G' + bf]
                        # in-AP iterated (d, h, fr=2G'+bf): strides
                        # d:1, h:16, fr: (G':64, bf:32) merge to 32x16
                        f0 = 16 * (4 * i + g)
                        pv = ps[:].rearrange(
                            "p (fr d) -> p d fr", d=CPC * 2)
                        # fr index = G'*2+bf <- psum col (G'*64 + bf*32)/32;
                        # (fr d) grouping: col = fr*32 + (h*16 + d) with
                        # h*16+d = b%2*16+d: psum col = G'*64+b*16+d
                        #        = G'*64 + bf*32 + h*16 + d  ✓
                        bv = bigx[:, :, :, f0:f0 + 16].rearrange(
                            "p d h fr -> p d (h fr)")
                        nc.scalar.copy(
                            bv.rearrange("p d x -> p (d x)"),
                            pv.rearrange(
                                "p d fr -> p (d fr)"
                            ).rearrange(
                                "p (d h fr) -> p (d (h fr))",
                                d=CPC, h=2),
                        )

                # ---- Phase B: folds -> DFT -> scan -> inverse+OLA -> tanh
                for d in range(CPC):
                    bx0 = bigx[:, d, 0, :]
                    bx1 = bigx[:, d, 1, :]
                    eh0 = eop.tile([128, 512], FP16, tag="eh0")
                    eh1 = eop.tile([128, 512], FP16, tag="eh1")
                    c1t = eop.tile([128, 512], FP16, tag="c1")
                    c2t = eop.tile([128, 512], FP16, tag="c2")
                    b0t = eop.tile([128, 512], FP16, tag="b0")
                    b1t = eop.tile([128, 512], FP16, tag="b1")
                    nc.vector.tensor_tensor(eh0[:], bx0[:, 0:512], bx0[:, 1:513], op=ADD)
                    nc.vector.tensor_tensor(eh1[:], bx1[:, 0:512], bx1[:, 1:513], op=ADD)
                    nc.vector.tensor_tensor(b0t[:], bx0[:, 0:512], bx0[:, 1:513], op=SUB)
                    nc.vector.tensor_tensor(b1t[:], bx1[:, 0:512], bx1[:, 1:513], op=SUB)
                    nc.vector.tensor_tensor(c1t[:], eh0[:], eh1[:], op=ADD)
                    nc.vector.tensor_tensor(c2t[:], eh0[:], eh1[:], op=SUB)
                    # (lhsT block, rhs tile) per m-chunk
                    plan = [
                        [(0, c1t)],
                        [(1, c2t)],
                        [(2, b0t), (3, b1t)],
                        [(4, b0t), (5, b1t)],
                    ]
                    us = []
                    for m in range(4):
                        ps = psp.tile([128, 512], FP32)
                        terms = plan[m]
                        for ti, (blk, src) in enumerate(terms):
                            nc.tensor.matmul(
                                ps[:],
                                lhsT=fsb[:, blk, :],
                                rhs=src[:],
                                start=(ti == 0),
                                stop=(ti == len(terms) - 1),
                            )
                        # col 0 zero pad keeps the shifted (f-1) OLA read
                        # 512 wide with a well-formed psum group
                        u = u_all[(d * 4 + m) % 8]
                        idx = d * 4 + m
                        nc.vector.tensor_tensor_scan(
                            u[:, 1:513], ps[:],
                            trsb[:, idx:idx + 1].broadcast_to((128, 512)),
                            0.0, op0=ADD, op1=MUL,
                        )
                        us.append(u)
                    # inverse DFT with overlap-add folded into PSUM:
                    # out col j=2f+s gets W_s[:,f] + W_{s+2}[:,f-1]
                    res = rp.tile([128, JCOLS], FP32)
                    ov = res[:].rearrange("p (f two) -> p two f", two=2)
                    for s01 in range(2):
                        pout = pwp.tile([128, 512], FP32)
                        for k in range(4):
                            nc.tensor.matmul(
                                pout[:],
                                lhsT=gsb[:, k, s01, :],
                                rhs=us[k][:, 1:513],
                                start=(k == 0),
                                stop=False,
                            )
                        for k in range(4):
                            nc.tensor.matmul(
                                pout[:],
                                lhsT=gsb[:, k, s01 + 2, :],
                                rhs=us[k][:, 0:512],
                                start=False,
                                stop=(k == 3),
                            )
                        nc.scalar.activation(
                            ov[:, s01, :], pout[:],
                            mybir.ActivationFunctionType.Tanh,
                            scale=gain_sb[:, d:d + 1],
                        )
                    nc.gpsimd.dma_start(out=out_d[d], in_=res[:])
    nc.compile()
    return nc


def build_in_maps(x, transfer, mixer_matrix, gain):
    f_blocks, g_l, newperm = _build_dft_matrices()

    # transfer per dof (re/im parts share the same real coefficient),
    # permuted into the chunked dof order
    tr_plain = np.empty((C, NDOF), np.float32)
    tr_plain[:, :NCOEF] = transfer
    tr_plain[:, NCOEF:] = transfer[:, 1:256]
    tr_dof = np.ascontiguousarray(tr_plain[:, newperm])

    in_maps = []
    for core in range(NCORES):
        b, h = core // 2, core % 2
        d0 = h * CPC
        mixcols = mixer_matrix[:, d0:d0 + CPC]               # [32, 16]
        mix4 = np.zeros((128, 4 * CPC), np.float16)
        for q in range(4):
            mix4[32 * q:32 * (q + 1), CPC * q:CPC * (q + 1)] = mixcols
        trd = tr_dof[d0:d0 + CPC]                            # [16, 512]
        trt = np.ascontiguousarray(
            trd.reshape(CPC, 4, 128).transpose(2, 0, 1).reshape(128, CPC * 4))
        # xq[32*bq + c, G*128 + p] = x[c, G*512 + bq*128 + p]
        xqv = np.ascontiguousarray(
            x[b].reshape(C, T // 512, 4, 128).transpose(2, 0, 1, 3)
            .reshape(128, T // 4).astype(np.float16))
        in_maps.append({
            "xq": xqv,
            "mix4": mix4,
            "fmat": f_blocks,
            "gmat": g_l,
            "trt": trt,
            "gainv": np.ascontiguousarray(gain[d0:d0 + CPC].reshape(1, CPC)),
        })
    return in_maps


_PROGRAM_CACHE = {}


def kernel(x, transfer, mixer_matrix, gain, **run_kwargs):
    x = np.ascontiguousarray(x, np.float32)
    transfer = np.asarray(transfer, np.float32)
    mixer_matrix = np.asarray(mixer_matrix, np.float32)
    gain = np.asarray(gain, np.float32)

    in_maps = build_in_maps(x, transfer, mixer_matrix, gain)

    if "nc" not in _PROGRAM_CACHE:
        _PROGRAM_CACHE["nc"] = _build_program()
    nc = _PROGRAM_CACHE["nc"]

    res = run_bass_kernel_spmd(nc, in_maps, list(range(NCORES)), **run_kwargs)

    out = np.empty((B, C, T), np.float32)
    for core in range(NCORES):
        b, h = core // 2, core % 2
        o = res.results[core]["out"]                    # [16, 128, 1024]
        out[b, h * CPC:(h + 1) * CPC] = o.transpose(0, 2, 1).reshape(CPC, T)
    kernel.last_results = res
    return out


# revision 10
# speedup vs baseline: 1.3839x; 1.2508x over previous
"""Trainium2 Bass kernel for nn_Block_9345848836513.

Per-core pipeline (8 cores = 4 batches x 2 channel-halves, 16 ch each):
  1. channel mix in fp16 on PE: lhsT = x-chunk [128 rows = 4 consecutive
     128-tau windows x 32 ch, 128 taus], rhs = block-diag 4x mixer
     [128, 64] -> psum [tau, (window, ch)]; 256 matmuls, full 128-row
     contraction, one psum tag.  x arrives from HBM as fp16 (halves the
     phase-A DMA, its critical path).  Copies scatter psum into BigX
     laid out [p, ch, j-parity, frame] (fp16) so later folds read
     contiguous rows.
  2. forward rfft of 512-sample frames (hop 256) via radix-4-folded real
     DFT: DVE folds produce C1/C2 (even-bin sources, 128 long) and
     B0/B1 (odd-bin halves); 6 fp16 matmuls per channel give all 512
     real DOFs (vs 16 for a dense [512,512]).
  3. per-frame recurrence out_i = (spec_i + out_{i-1}) * transfer via
     tensor_tensor_scan along the frame axis (fp32 state, fp16 out)
  4. inverse rfft with Hann folded into the fp16 matrix; overlap-add
     folded into PSUM accumulation (second matmul group reads with a
     one-column shift); tanh straight from PSUM.
BigX is double-buffered across reps so rep k+1's DMA/mix overlaps rep
k's transform phase; psum budget 2 (mix) + 3 (fwd) + 3 (inv) = 8 banks.
Output DMAs ride the Pool SWDGE queue, x loads the SP HWDGE queue.
"""

import numpy as np

import concourse.bass as bass
import concourse.mybir as mybir
import concourse.tile as tile
from concourse import bacc
from concourse.bass_utils import run_bass_kernel_spmd

WINDOW = 512
HOP = 256
NCOEF = 257
NDOF = 512
B, C, T = 4, 32, 131072
F = T // HOP          # 512 frames
CPC = 16              # channels per core
NCORES = 8
JCOLS = T // 128      # 1024 output columns per channel
FPAD = F + 1          # 513 frame slots per (ch, parity); last is zero pad
FP32 = mybir.dt.float32
FP32R = mybir.dt.float32r
FP16 = mybir.dt.float16
U16 = mybir.dt.uint16


def _build_dft_matrices():
    w = np.arange(WINDOW)
    k = np.arange(NCOEF)
    ang = 2.0 * np.pi * np.outer(w, k) / WINDOW
    cos, sin = np.cos(ang), np.sin(ang)
    fmat = np.zeros((WINDOW, NDOF), np.float64)
    fmat[:, :NCOEF] = cos
    fmat[:, NCOEF:] = -sin[:, 1:256]
    hann = 0.5 - 0.5 * np.cos(2.0 * np.pi * w / WINDOW)
    g = np.zeros((NDOF, WINDOW), np.float64)
    g[0, :] = 1.0
    g[256, :] = cos[:, 256]
    for kk in range(1, 256):
        g[kk, :] = 2.0 * cos[:, kk]
        g[256 + kk, :] = -2.0 * sin[:, kk]
    g *= hann[None, :] / WINDOW

    # dof indexing in the plain layout: Re k -> k (0..256), Im k -> 256+k
    def dofs_re(ks):
        return list(ks)

    def dofs_im(ks):
        return [256 + kk for kk in ks if 1 <= kk <= 255]

    # 4 chunks of 128 dofs: bins k=0 mod 4 / 2 mod 4 / 1 mod 4 / 3 mod 4.
    # Sources after the DVE folds (x split in 128-quarters x0..x3 of the
    # 512 window): C1 = x0+x1+x2+x3, C2 = x0-x1+x2-x3 (via e-halves),
    # B0 = x0-x2, B1 = x1-x3.  For k=0 mod 4: X[k] = sum_n C1[n] W^nk;
    # k=2 mod 4: C2; odd k: X[k] = sum_n B0[n] W^nk + B1[n] W^(n+128)k.
    # All coefficients are rows of the plain fmat.
    chunk_k = [
        list(range(0, 257, 4)),
        list(range(2, 256, 4)),
        list(range(1, 256, 4)),
        list(range(3, 256, 4)),
    ]
    newperm = []
    for ks in chunk_k:
        newperm += dofs_re(ks) + dofs_im(ks)
    newperm = np.asarray(newperm)
    assert newperm.size == NDOF and np.unique(newperm).size == NDOF

    p0, p1, p2, p3 = (newperm[128 * i:128 * (i + 1)] for i in range(4))
    # 6 lhsT blocks: (C1->chunk0, C2->chunk1, B0->c2, B1->c2, B0->c3, B1->c3)
    f_blocks = np.stack([
        fmat[0:128, p0],
        fmat[0:128, p1],
        fmat[0:128, p2],
        fmat[128:256, p2],
        fmat[0:128, p3],
        fmat[128:256, p3],
    ], axis=1)                                        # [128, 6, 128]
    g2 = g[newperm, :]                                # [512, 512]
    g_l = g2.reshape(4, 128, 4, 128).transpose(1, 0, 2, 3)
    return (np.ascontiguousarray(f_blocks, np.float16),
            np.ascontiguousarray(g_l, np.float16), newperm)


def _build_program(detect_races=True, reps=1):
    nc = bacc.Bacc("TRN2", target_bir_lowering=False, num_devices=NCORES,
                   detect_race_conditions=detect_races)
    # xq[32*b + c, G*128 + p] = x[c, G*512 + b*128 + p]  (fp16, host layout)
    xq = nc.dram_tensor("xq", [128, T // 4], FP16, kind="ExternalInput")
    # mix4[32*b + c, 16*b + d] = mixer[c, d0+d]; zero elsewhere (fp16)
    mix4 = nc.dram_tensor("mix4", [128, 4 * CPC], FP16, kind="ExternalInput")
    fmat = nc.dram_tensor("fmat", [128, 6, 128], FP16, kind="ExternalInput")
    gmat = nc.dram_tensor("gmat", [128, 4, 4, 128], FP16, kind="ExternalInput")
    trt = nc.dram_tensor("trt", [128, CPC * 4], FP32, kind="ExternalInput")
    gainv = nc.dram_tensor("gainv", [1, CPC], FP32, kind="ExternalInput")
    out_d = nc.dram_tensor("out", [CPC, 128, JCOLS], FP32, kind="ExternalOutput")

    ADD, MUL = mybir.AluOpType.add, mybir.AluOpType.mult
    SUB = mybir.AluOpType.subtract

    with tile.TileContext(nc) as tc:
        with (
            tc.tile_pool(name="singles", bufs=1) as singles,
            tc.tile_pool(name="xa", bufs=3) as xa,
            tc.tile_pool(name="pmix", bufs=2, space="PSUM") as pmix,
            tc.tile_pool(name="eo", bufs=3) as eop,
            tc.tile_pool(name="rp", bufs=3) as rp,
            tc.tile_pool(name="psp", bufs=3, space="PSUM") as psp,
            tc.tile_pool(name="pwp", bufs=3, space="PSUM") as pwp,
        ):
            fsb = singles.tile([128, 6, 128], FP16)
            gsb = singles.tile([128, 4, 4, 128], FP16)
            mix_sb = singles.tile([128, 4 * CPC], FP16)
            trsb = singles.tile([128, CPC * 4], FP32)
            gain_sb = singles.tile([128, CPC], FP32)
            # bigx[p, d, h, f] = y[d, t = 256f + 128h + p]  (j = 2f+h)
            bigxs = [singles.tile([128, CPC, 2, FPAD], FP16, name=f"bigx{r}")
                     for r in range(2)]
            u_all = [singles.tile([128, 513], FP16, name=f"uall{i}")
                     for i in range(8)]
            for ut in u_all:
                nc.vector.memset(ut[:, 0:1].bitcast(U16), 0)
            # params ride the SWDGE queue so the first phase-A x-load
            # (HWDGE) isn't queued behind the parameter DMAs
            nc.sync.dma_start(out=mix_sb[:], in_=mix4[:])
            nc.gpsimd.dma_start(out=fsb[:], in_=fmat[:])
            nc.gpsimd.dma_start(out=gsb[:], in_=gmat[:])
            nc.gpsimd.dma_start(out=trsb[:], in_=trt[:])
            nc.gpsimd.dma_start(out=gain_sb[:], in_=gainv[:].to_broadcast((128, CPC)))
            for bx in bigxs:
                nc.vector.memset(bx[:, :, :, F:FPAD].bitcast(U16), 0)

            for _rep in range(reps):
                bigx = bigxs[_rep % 2]
                # ---- Phase A: transposed mix into BigX (fp16) ----
                # xt tile i holds groups G in [32i, 32(i+1)); matmul for
                # group G: lhsT = xt[:, local window] -> psum cols
                # [64*G' + 16*b + d] with j = 4G + b, h = b%2,
                # f = 2*(8s + G') + b//2  (s = psum slab index 4i+g).
                for i in range(8):
                    xt = xa.tile([128, 4096], FP16, tag="xt")
                    nc.sync.dma_start(
                        out=xt[:], in_=xq[:, 4096 * i:4096 * (i + 1)])
                    for g in range(4):
                        ps = pmix.tile([128, 512], FP32, tag="pmix")
                        for gp in range(8):
                            nc.tensor.matmul(
                                ps[:, 64 * gp:64 * (gp + 1)],
                                lhsT=xt[:, 1024 * g + 128 * gp:
                                        1024 * g + 128 * (gp + 1)],
                                rhs=mix_sb[:],
                            )
                        # psum col = G'*64 + b*16 + d, with b = 2*bf + h
                        # and frame offset fr = 2*G' + bf, i.e.
                        # col = 32*fr + 16*h + d -> "(fr h d)" grouping.
                        f0 = 16 * (4 * i + g)
                        nc.scalar.copy(
                            bigx[:, :, :, f0:f0 + 16],
                            ps[:].rearrange("p (fr h d) -> p d h fr",
                                            fr=16, h=2),
                        )

                # ---- Phase B: folds -> DFT -> scan -> inverse+OLA -> tanh
                for d in range(CPC):
                    bx0 = bigx[:, d, 0, :]
                    bx1 = bigx[:, d, 1, :]
                    eh0 = eop.tile([128, 512], FP16, tag="eh0")
                    eh1 = eop.tile([128, 512], FP16, tag="eh1")
                    c1t = eop.tile([128, 512], FP16, tag="c1")
                    c2t = eop.tile([128, 512], FP16, tag="c2")
                    b0t = eop.tile([128, 512], FP16, tag="b0")
                    b1t = eop.tile([128, 512], FP16, tag="b1")
                    nc.vector.tensor_tensor(eh0[:], bx0[:, 0:512], bx0[:, 1:513], op=ADD)
                    nc.vector.tensor_tensor(eh1[:], bx1[:, 0:512], bx1[:, 1:513], op=ADD)
                    nc.vector.tensor_tensor(c1t[:], eh0[:], eh1[:], op=ADD)
                    nc.vector.tensor_tensor(c2t[:], eh0[:], eh1[:], op=SUB)
                    # odd-bin folds ride the Pool engine: they're consumed
                    # by the later m2/m3 matmuls, and DVE is scan-bound
                    nc.gpsimd.tensor_tensor(b0t[:], bx0[:, 0:512], bx0[:, 1:513], op=SUB)
                    nc.gpsimd.tensor_tensor(b1t[:], bx1[:, 0:512], bx1[:, 1:513], op=SUB)
                    # (lhsT block, rhs tile) per m-chunk
                    plan = [
                        [(0, c1t)],
                        [(1, c2t)],
                        [(2, b0t), (3, b1t)],
                        [(4, b0t), (5, b1t)],
                    ]
                    us = []
                    for m in range(4):
                        ps = psp.tile([128, 512], FP32)
                        terms = plan[m]
                        for ti, (blk, src) in enumerate(terms):
                            nc.tensor.matmul(
                                ps[:],
                                lhsT=fsb[:, blk, :],
                                rhs=src[:],
                                start=(ti == 0),
                                stop=(ti == len(terms) - 1),
                            )
                        # col 0 zero pad keeps the shifted (f-1) OLA read
                        # 512 wide with a well-formed psum group
                        u = u_all[(d * 4 + m) % 8]
                        idx = d * 4 + m
                        nc.vector.tensor_tensor_scan(
                            u[:, 1:513], ps[:],
                            trsb[:, idx:idx + 1].broadcast_to((128, 512)),
                            0.0, op0=ADD, op1=MUL,
                        )
                        us.append(u)
                    # inverse DFT with overlap-add folded into PSUM:
                    # out col j=2f+s gets W_s[:,f] + W_{s+2}[:,f-1]
                    res = rp.tile([128, JCOLS], FP32)
                    ov = res[:].rearrange("p (f two) -> p two f", two=2)
                    for s01 in range(2):
                        pout = pwp.tile([128, 512], FP32)
                        for k in range(4):
                            nc.tensor.matmul(
                                pout[:],
                                lhsT=gsb[:, k, s01, :],
                                rhs=us[k][:, 1:513],
                                start=(k == 0),
                                stop=False,
                            )
                        for k in range(4):
                            nc.tensor.matmul(
                                pout[:],
                                lhsT=gsb[:, k, s01 + 2, :],
                                rhs=us[k][:, 0:512],
                                start=False,
                                stop=(k == 3),
                            )
                        nc.scalar.activation(
                            ov[:, s01, :], pout[:],
                            mybir.ActivationFunctionType.Tanh,
                            scale=gain_sb[:, d:d + 1],
                        )
                    nc.gpsimd.dma_start(out=out_d[d], in_=res[:])
    nc.compile()
    return nc


def build_in_maps(x, transfer, mixer_matrix, gain):
    f_blocks, g_l, newperm = _build_dft_matrices()

    # transfer per dof (re/im parts share the same real coefficient),
    # permuted into the chunked dof order
    tr_plain = np.empty((C, NDOF), np.float32)
    tr_plain[:, :NCOEF] = transfer
    tr_plain[:, NCOEF:] = transfer[:, 1:256]
    tr_dof = np.ascontiguousarray(tr_plain[:, newperm])

    in_maps = []
    for core in range(NCORES):
        b, h = core // 2, core % 2
        d0 = h * CPC
        mixcols = mixer_matrix[:, d0:d0 + CPC]               # [32, 16]
        mix4 = np.zeros((128, 4 * CPC), np.float16)
        for q in range(4):
            mix4[32 * q:32 * (q + 1), CPC * q:CPC * (q + 1)] = mixcols
        trd = tr_dof[d0:d0 + CPC]                            # [16, 512]
        trt = np.ascontiguousarray(
            trd.reshape(CPC, 4, 128).transpose(2, 0, 1).reshape(128, CPC * 4))
        # xq[32*bq + c, G*128 + p] = x[c, G*512 + bq*128 + p]
        xqv = np.ascontiguousarray(
            x[b].reshape(C, T // 512, 4, 128).transpose(2, 0, 1, 3)
            .reshape(128, T // 4).astype(np.float16))
        in_maps.append({
            "xq": xqv,
            "mix4": mix4,
            "fmat": f_blocks,
            "gmat": g_l,
            "trt": trt,
            "gainv": np.ascontiguousarray(gain[d0:d0 + CPC].reshape(1, CPC)),
        })
    return in_maps


_PROGRAM_CACHE = {}


def kernel(x, transfer, mixer_matrix, gain, **run_kwargs):
    x = np.ascontiguousarray(x, np.float32)
    transfer = np.asarray(transfer, np.float32)
    mixer_matrix = np.asarray(mixer_matrix, np.float32)
    gain = np.asarray(gain, np.float32)

    in_maps = build_in_maps(x, transfer, mixer_matrix, gain)

    if "nc" not in _PROGRAM_CACHE:
        _PROGRAM_CACHE["nc"] = _build_program()
    nc = _PROGRAM_CACHE["nc"]

    res = run_bass_kernel_spmd(nc, in_maps, list(range(NCORES)), **run_kwargs)

    out = np.empty((B, C, T), np.float32)
    for core in range(NCORES):
        b, h = core // 2, core % 2
        o = res.results[core]["out"]                    # [16, 128, 1024]
        out[b, h * CPC:(h + 1) * CPC] = o.transpose(0, 2, 1).reshape(CPC, T)
    kernel.last_results = res
    return out


# revision 21
# speedup vs baseline: 3.2271x; 2.3320x over previous
"""Trainium2 Bass kernel for nn_Block_9345848836513.

Per-core pipeline (8 cores = 4 batches x 2 channel-halves, 16 ch each):
  1. channel mix in fp16 on PE: lhsT = x-chunk [128 rows = 4 consecutive
     128-tau windows x 32 ch, 128 taus], rhs = block-diag 4x mixer
     [128, 64] -> psum [tau, (window, ch)]; 256 matmuls, full 128-row
     contraction, one psum tag.  x arrives from HBM as fp16 (halves the
     phase-A DMA, its critical path).  Copies scatter psum into BigX
     laid out [p, ch, j-parity, frame] (fp16) so later folds read
     contiguous rows.
  2. forward rfft of 512-sample frames (hop 256) via radix-4-folded real
     DFT: DVE folds produce C1/C2 (even-bin sources, 128 long) and
     B0/B1 (odd-bin halves); 6 fp16 matmuls per channel give all 512
     real DOFs (vs 16 for a dense [512,512]).
  3. per-frame recurrence out_i = (spec_i + out_{i-1}) * transfer via
     tensor_tensor_scan along the frame axis (fp32 state, fp16 out)
  4. inverse rfft with Hann folded into the fp16 matrix; overlap-add
     folded into PSUM accumulation (second matmul group reads with a
     one-column shift); tanh straight from PSUM.
BigX is double-buffered across reps so rep k+1's DMA/mix overlaps rep
k's transform phase; psum budget 2 (mix) + 3 (fwd) + 3 (inv) = 8 banks.
Output DMAs ride the Pool SWDGE queue, x loads the SP HWDGE queue.
"""

import numpy as np

import concourse.bass as bass
import concourse.mybir as mybir
import concourse.tile as tile
from concourse import bacc
from concourse.bass_utils import run_bass_kernel_spmd

WINDOW = 512
HOP = 256
NCOEF = 257
NDOF = 512
B, C, T = 4, 32, 131072
F = T // HOP          # 512 frames
CPC = 16              # channels per core
NCORES = 8
JCOLS = T // 128      # 1024 output columns per channel
FPAD = F + 1          # 513 frame slots per (ch, parity); last is zero pad
FP32 = mybir.dt.float32
FP32R = mybir.dt.float32r
FP16 = mybir.dt.float16
U16 = mybir.dt.uint16


def _build_dft_matrices():
    w = np.arange(WINDOW)
    k = np.arange(NCOEF)
    ang = 2.0 * np.pi * np.outer(w, k) / WINDOW
    cos, sin = np.cos(ang), np.sin(ang)
    fmat = np.zeros((WINDOW, NDOF), np.float64)
    fmat[:, :NCOEF] = cos
    fmat[:, NCOEF:] = -sin[:, 1:256]
    hann = 0.5 - 0.5 * np.cos(2.0 * np.pi * w / WINDOW)
    g = np.zeros((NDOF, WINDOW), np.float64)
    g[0, :] = 1.0
    g[256, :] = cos[:, 256]
    for kk in range(1, 256):
        g[kk, :] = 2.0 * cos[:, kk]
        g[256 + kk, :] = -2.0 * sin[:, kk]
    g *= hann[None, :] / WINDOW

    # dof indexing in the plain layout: Re k -> k (0..256), Im k -> 256+k
    def dofs_re(ks):
        return list(ks)

    def dofs_im(ks):
        return [256 + kk for kk in ks if 1 <= kk <= 255]

    # 4 chunks of 128 dofs: bins k=0 mod 4 / 2 mod 4 / 1 mod 4 / 3 mod 4.
    # Sources after the DVE folds (x split in 128-quarters x0..x3 of the
    # 512 window): C1 = x0+x1+x2+x3, C2 = x0-x1+x2-x3 (via e-halves),
    # B0 = x0-x2, B1 = x1-x3.  For k=0 mod 4: X[k] = sum_n C1[n] W^nk;
    # k=2 mod 4: C2; odd k: X[k] = sum_n B0[n] W^nk + B1[n] W^(n+128)k.
    # All coefficients are rows of the plain fmat.
    chunk_k = [
        list(range(0, 257, 4)),
        list(range(2, 256, 4)),
        list(range(1, 256, 4)),
        list(range(3, 256, 4)),
    ]
    newperm = []
    for ks in chunk_k:
        newperm += dofs_re(ks) + dofs_im(ks)
    newperm = np.asarray(newperm)
    assert newperm.size == NDOF and np.unique(newperm).size == NDOF

    p0, p1, p2, p3 = (newperm[128 * i:128 * (i + 1)] for i in range(4))
    # 6 lhsT blocks: (C1->chunk0, C2->chunk1, B0->c2, B1->c2, B0->c3, B1->c3)
    f_blocks = np.stack([
        fmat[0:128, p0],
        fmat[0:128, p1],
        fmat[0:128, p2],
        fmat[128:256, p2],
        fmat[0:128, p3],
        fmat[128:256, p3],
    ], axis=1)                                        # [128, 6, 128]
    g2 = g[newperm, :]                                # [512, 512]
    g_l = g2.reshape(4, 128, 4, 128).transpose(1, 0, 2, 3)
    return (np.ascontiguousarray(f_blocks, np.float16),
            np.ascontiguousarray(g_l, np.float16), newperm)


def _build_program(detect_races=True, reps=1, lag=0, psp_bufs=3, pwp_bufs=3, pool_folds=False, nu=12):
    nc = bacc.Bacc("TRN2", target_bir_lowering=False, num_devices=NCORES,
                   detect_race_conditions=detect_races)
    # xq[32*b + c, G*128 + p] = x[c, G*512 + b*128 + p]  (fp16, host layout)
    xq = nc.dram_tensor("xq", [128, T // 4], FP16, kind="ExternalInput")
    # mix4[32*b + c, 16*b + d] = mixer[c, d0+d]; zero elsewhere (fp16)
    mix4 = nc.dram_tensor("mix4", [128, 4 * CPC], FP16, kind="ExternalInput")
    fmat = nc.dram_tensor("fmat", [128, 6, 128], FP16, kind="ExternalInput")
    gmat = nc.dram_tensor("gmat", [128, 4, 4, 128], FP16, kind="ExternalInput")
    trt = nc.dram_tensor("trt", [128, CPC * 4], FP32, kind="ExternalInput")
    gainv = nc.dram_tensor("gainv", [1, CPC], FP32, kind="ExternalInput")
    out_d = nc.dram_tensor("out", [CPC, 128, JCOLS], FP32, kind="ExternalOutput")

    ADD, MUL = mybir.AluOpType.add, mybir.AluOpType.mult
    SUB = mybir.AluOpType.subtract

    with tile.TileContext(nc) as tc:
        with (
            tc.tile_pool(name="singles", bufs=1) as singles,
            tc.tile_pool(name="xa", bufs=3) as xa,
            tc.tile_pool(name="pmix", bufs=2, space="PSUM") as pmix,
            tc.tile_pool(name="eo", bufs=3) as eop,
            tc.tile_pool(name="rp", bufs=3) as rp,
            tc.tile_pool(name="psp", bufs=psp_bufs, space="PSUM") as psp,
            tc.tile_pool(name="pwp", bufs=pwp_bufs, space="PSUM") as pwp,
        ):
            fsb = singles.tile([128, 6, 128], FP16)
            gsb = singles.tile([128, 4, 4, 128], FP16)
            mix_sb = singles.tile([128, 4 * CPC], FP16)
            trsb = singles.tile([128, CPC * 4], FP32)
            gain_sb = singles.tile([128, CPC], FP32)
            # bigx[p, d, h, f] = y[d, t = 256f + 128h + p]  (j = 2f+h)
            bigxs = [singles.tile([128, CPC, 2, FPAD], FP16, name=f"bigx{r}")
                     for r in range(2)]
            NU = nu
            u_all = [singles.tile([128, 513], FP16, name=f"uall{i}")
                     for i in range(NU)]
            for ut in u_all:
                nc.vector.memset(ut[:, 0:1].bitcast(U16), 0)
            # params ride the SWDGE queue so the first phase-A x-load
            # (HWDGE) isn't queued behind the parameter DMAs
            nc.sync.dma_start(out=mix_sb[:], in_=mix4[:])
            nc.gpsimd.dma_start(out=fsb[:], in_=fmat[:])
            nc.gpsimd.dma_start(out=gsb[:], in_=gmat[:])
            nc.gpsimd.dma_start(out=trsb[:], in_=trt[:])
            nc.gpsimd.dma_start(out=gain_sb[:], in_=gainv[:].to_broadcast((128, CPC)))
            for bx in bigxs:
                nc.vector.memset(bx[:, :, :, F:FPAD].bitcast(U16), 0)

            def emit_a_tile(bigx, i):
                # ---- Phase A: transposed mix into BigX (fp16) ----
                # xt tile i holds groups G in [32i, 32(i+1)); matmul for
                # group G: lhsT = xt[:, local window] -> psum cols
                # [64*G' + 16*b + d] with j = 4G + b, h = b%2,
                # f = 2*(8s + G') + b//2  (s = psum slab index 4i+g).
                xt = xa.tile([128, 4096], FP16, tag="xt")
                nc.sync.dma_start(
                    out=xt[:], in_=xq[:, 4096 * i:4096 * (i + 1)])
                for g in range(4):
                    ps = pmix.tile([128, 512], FP32, tag="pmix")
                    for gp in range(8):
                        nc.tensor.matmul(
                            ps[:, 64 * gp:64 * (gp + 1)],
                            lhsT=xt[:, 1024 * g + 128 * gp:
                                    1024 * g + 128 * (gp + 1)],
                            rhs=mix_sb[:],
                        )
                    # psum col = G'*64 + b*16 + d, with b = 2*bf + h
                    # and frame offset fr = 2*G' + bf, i.e.
                    # col = 32*fr + 16*h + d -> "(fr h d)" grouping.
                    f0 = 16 * (4 * i + g)
                    nc.scalar.copy(
                        bigx[:, :, :, f0:f0 + 16],
                        ps[:].rearrange("p (fr h d) -> p d h fr",
                                        fr=16, h=2),
                    )

            for _rep in range(reps):
                bigx = bigxs[_rep % 2]
                if _rep == 0:
                    for i in range(8):
                        emit_a_tile(bigx, i)
                nxt = bigxs[(_rep + 1) % 2]

                # ---- Phase B: folds -> DFT -> scan -> inverse+OLA -> tanh
                # The next rep's phase A (DMA + mix + copies into the other
                # BigX buffer) is interleaved one i-tile per two channels so
                # the in-order engine queues never see a rep-boundary stall.
                LAG = lag
                us_by_d = {}
                for dd in range(CPC + LAG):
                  if dd < CPC and dd % 2 == 1 and _rep + 1 < reps:
                    emit_a_tile(nxt, dd // 2)
                  if dd < CPC:
                    d = dd
                    bx0 = bigx[:, d, 0, :]
                    bx1 = bigx[:, d, 1, :]
                    eh0 = eop.tile([128, 512], FP16, tag="eh0")
                    eh1 = eop.tile([128, 512], FP16, tag="eh1")
                    c1t = eop.tile([128, 512], FP16, tag="c1")
                    c2t = eop.tile([128, 512], FP16, tag="c2")
                    b0t = eop.tile([128, 512], FP16, tag="b0")
                    b1t = eop.tile([128, 512], FP16, tag="b1")
                    nc.vector.tensor_tensor(eh0[:], bx0[:, 0:512], bx0[:, 1:513], op=ADD)
                    nc.vector.tensor_tensor(eh1[:], bx1[:, 0:512], bx1[:, 1:513], op=ADD)
                    nc.vector.tensor_tensor(c1t[:], eh0[:], eh1[:], op=ADD)
                    nc.vector.tensor_tensor(c2t[:], eh0[:], eh1[:], op=SUB)
                    # odd-bin folds ride the Pool engine: they're consumed
                    # by the later m2/m3 matmuls, and DVE is scan-bound
                    (nc.gpsimd if pool_folds else nc.vector).tensor_tensor(b0t[:], bx0[:, 0:512], bx0[:, 1:513], op=SUB)
                    (nc.gpsimd if pool_folds else nc.vector).tensor_tensor(b1t[:], bx1[:, 0:512], bx1[:, 1:513], op=SUB)
                    # (lhsT block, rhs tile) per m-chunk
                    plan = [
                        [(0, c1t)],
                        [(1, c2t)],
                        [(2, b0t), (3, b1t)],
                        [(4, b0t), (5, b1t)],
                    ]
                    us = []
                    for m in range(4):
                        ps = psp.tile([128, 512], FP32)
                        terms = plan[m]
                        for ti, (blk, src) in enumerate(terms):
                            nc.tensor.matmul(
                                ps[:],
                                lhsT=fsb[:, blk, :],
                                rhs=src[:],
                                start=(ti == 0),
                                stop=(ti == len(terms) - 1),
                            )
                        # col 0 zero pad keeps the shifted (f-1) OLA read
                        # 512 wide with a well-formed psum group
                        u = u_all[(d * 4 + m) % NU]
                        idx = d * 4 + m
                        nc.vector.tensor_tensor_scan(
                            u[:, 1:513], ps[:],
                            trsb[:, idx:idx + 1].broadcast_to((128, 512)),
                            0.0, op0=ADD, op1=MUL,
                        )
                        us.append(u)
                    us_by_d[d] = us
                  if dd >= LAG:
                    d = dd - LAG
                    us = us_by_d.pop(d)
                    # inverse DFT with overlap-add folded into PSUM:
                    # out col j=2f+s gets W_s[:,f] + W_{s+2}[:,f-1]
                    res = rp.tile([128, JCOLS], FP32)
                    ov = res[:].rearrange("p (f two) -> p two f", two=2)
                    for s01 in range(2):
                        pout = pwp.tile([128, 512], FP32)
                        for k in range(4):
                            nc.tensor.matmul(
                                pout[:],
                                lhsT=gsb[:, k, s01, :],
                                rhs=us[k][:, 1:513],
                                start=(k == 0),
                                stop=False,
                            )
                        for k in range(4):
                            nc.tensor.matmul(
                                pout[:],
                                lhsT=gsb[:, k, s01 + 2, :],
                                rhs=us[k][:, 0:512],
                                start=False,
                                stop=(k == 3),
                            )
                        nc.scalar.activation(
                            ov[:, s01, :], pout[:],
                            mybir.ActivationFunctionType.Tanh,
                            scale=gain_sb[:, d:d + 1],
                        )
                    nc.gpsimd.dma_start(out=out_d[d], in_=res[:])
    nc.compile()
    return nc


def build_in_maps(x, transfer, mixer_matrix, gain):
    f_blocks, g_l, newperm = _build_dft_matrices()

    # transfer per dof (re/im parts share the same real coefficient),
    # permuted into the chunked dof order
    tr_plain = np.empty((C, NDOF), np.float32)
    tr_plain[:, :NCOEF] = transfer
    tr_plain[:, NCOEF:] = transfer[:, 1:256]
    tr_dof = np.ascontiguousarray(tr_plain[:, newperm])

    in_maps = []
    for core in range(NCORES):
        b, h = core // 2, core % 2
        d0 = h * CPC
        mixcols = mixer_matrix[:, d0:d0 + CPC]               # [32, 16]
        mix4 = np.zeros((128, 4 * CPC), np.float16)
        for q in range(4):
            mix4[32 * q:32 * (q + 1), CPC * q:CPC * (q + 1)] = mixcols
        trd = tr_dof[d0:d0 + CPC]                            # [16, 512]
        trt = np.ascontiguousarray(
            trd.reshape(CPC, 4, 128).transpose(2, 0, 1).reshape(128, CPC * 4))
        # xq[32*bq + c, G*128 + p] = x[c, G*512 + bq*128 + p]
        xqv = np.ascontiguousarray(
            x[b].reshape(C, T // 512, 4, 128).transpose(2, 0, 1, 3)
            .reshape(128, T // 4).astype(np.float16))
        in_maps.append({
            "xq": xqv,
            "mix4": mix4,
            "fmat": f_blocks,
            "gmat": g_l,
            "trt": trt,
            "gainv": np.ascontiguousarray(gain[d0:d0 + CPC].reshape(1, CPC)),
        })
    return in_maps


_PROGRAM_CACHE = {}


def kernel(x, transfer, mixer_matrix, gain, **run_kwargs):
    x = np.ascontiguousarray(x, np.float32)
    transfer = np.asarray(transfer, np.float32)
    mixer_matrix = np.asarray(mixer_matrix, np.float32)
    gain = np.asarray(gain, np.float32)

    in_maps = build_in_maps(x, transfer, mixer_matrix, gain)

    if "nc" not in _PROGRAM_CACHE:
        _PROGRAM_CACHE["nc"] = _build_program()
    nc = _PROGRAM_CACHE["nc"]

    res = run_bass_kernel_spmd(nc, in_maps, list(range(NCORES)), **run_kwargs)

    out = np.empty((B, C, T), np.float32)
    for core in range(NCORES):
        b, h = core // 2, core % 2
        o = res.results[core]["out"]                    # [16, 128, 1024]
        out[b, h * CPC:(h + 1) * CPC] = o.transpose(0, 2, 1).reshape(CPC, T)
    kernel.last_results = res
    return out
